# revision 31
# baseline (speedup 1.0000x reference)
"""Multi-head attention (B=2, N=2048, C=1024, H=16) on 8 Trainium2 NeuronCores.

Sharding: tensor-parallel over heads (2 heads/core) for qkv-proj + attention;
all-to-all of the attention output (split into 4 half-batch collectives,
pipelined under attention), then each core runs the output projection over
the full channel dim for its token slices.  Host concatenates slices.

v2 changes vs baseline:
  - x is transposed on the HOST: xt input is [C, T] bf16, loaded with plain
    DMAs (split across the two HWDGE queues) -- no on-device DMA-transposes.
  - PE warmup stream at t=0 so HAM is at K=8/8 when real matmuls start.
  - q^T/k^T stored as single [128, tok] tiles (head A rows 0:64, head B rows
    64:128, the natural qkv-matmul output); the S^T matmuls are issued as
    K=64 pairs on disjoint PE row-groups (tile_position auto-derived), so
    both heads' S run CONCURRENTLY on the array halves -> S cost halves.
  - normalization uses reciprocal_approx_fast (~5x faster than reciprocal)
    and multiplies against the PSUM broadcast directly.
  - a2a staging/gather DMAs consolidated via rearrange APs (2+1 per stage
    instead of 16+8).

Device layouts (per core, heads A=2c, B=2c+1):
  x^T  [c_in, tok]          8 chunk tiles, plain DMA from host-transposed xt
  q^T/k^T [128, tok]        rows 0-63 head A, 64-127 head B
  v^T  [128, tok]           -> PE-transposed per 128-chunk into vn [128,130]
                            cols: [v_A|1|v_B|1]  (ones col -> softmax denom)
  S^T  [j, i] = k^T.T @ q^T per head (K=64 concurrent row-group pair)
  expS = exp(0.125 * S^T)   (ScalarE, bf16)
  out_u^T [65, i] = [v|1].T @ expS  (rows 0-63 out, row 64 = denominator)
  normalize via DVE reciprocal_approx_fast + fp32 K=1 broadcast matmul
"""

import numpy as np
import ml_dtypes
from contextlib import ExitStack

import concourse.bass as bass
import concourse.tile as tile
from concourse import bacc, mybir
from concourse.bass_utils import run_bass_kernel_spmd
from concourse.masks import make_identity

BF16 = mybir.dt.bfloat16
F32 = mybir.dt.float32
EXP = mybir.ActivationFunctionType.Exp
NPBF16 = ml_dtypes.bfloat16

NCORES = 8
B, NSEQ, C, H, D = 2, 2048, 1024, 16, 64
T = B * NSEQ                 # 4096 flattened tokens
SCALE = D ** -0.5            # folded into the exp activation
NKC = C // 128               # 8 contraction chunks
ITILE = 512                  # query tile (free dim of S^T)
NI = NSEQ // ITILE           # 4 i-tiles per batch
NJ = NSEQ // 128             # 16 key chunks per batch
JG = 2                       # j-chunks per exp group ([128,1024] psum tiles)
HALF = 1024                  # tokens per all-to-all (half batch)
TFRAG = HALF // NCORES       # 128 tokens per core per all-to-all
TSL = B * NSEQ // NCORES     # 512 output tokens per core

# a2a stages: (batch, start token within batch, n tokens); the final stages
# are small so the last collective's latency tail is short.
STAGES = [(0, 0, 1024), (0, 1024, 1024), (1, 0, 1024),
          (1, 1024, 512), (1, 1536, 512)]

DEBUG = False


def build_program():
    nc = bacc.Bacc("TRN2", target_bir_lowering=False, debug=False,
                   num_devices=NCORES)

    xt_d = nc.dram_tensor("xt", [C, T], BF16, kind="ExternalInput")
    wqk_d = nc.dram_tensor("wqk", [C, 256], BF16, kind="ExternalInput")
    wv_d = nc.dram_tensor("wv", [C, 128], BF16, kind="ExternalInput")
    wp_d = nc.dram_tensor("wproj", [C, C], BF16, kind="ExternalInput")
    bp_d = nc.dram_tensor("bproj", [1, C], BF16, kind="ExternalInput")
    y_d = nc.dram_tensor("y", [TSL, C], F32, kind="ExternalOutput")

    a2a_in = [nc.dram_tensor(f"a2a_in{q}", [NCORES * 128, nt // NCORES], BF16)
              for q, (_, _, nt) in enumerate(STAGES)]
    a2a_out = [nc.dram_tensor(f"a2a_out{q}", [NCORES * 128, nt // NCORES],
                              BF16)
               for q, (_, _, nt) in enumerate(STAGES)]
    if DEBUG:
        dbg = {
            "dqz": nc.dram_tensor("dqz", [128, NSEQ], BF16,
                                  kind="ExternalOutput"),
            "dkz": nc.dram_tensor("dkz", [128, NSEQ], BF16,
                                  kind="ExternalOutput"),
            "dvt": nc.dram_tensor("dvt", [128, NSEQ], BF16,
                                  kind="ExternalOutput"),
            "dvn": nc.dram_tensor("dvn", [128, 130], BF16,
                                  kind="ExternalOutput"),
            "dex": nc.dram_tensor("dex", [128, 2 * ITILE], BF16,
                                  kind="ExternalOutput"),
            "dou": nc.dram_tensor("dou", [65, ITILE], F32,
                                  kind="ExternalOutput"),
            "dot": nc.dram_tensor("dot", [64, NSEQ], BF16,
                                  kind="ExternalOutput"),
            "dos": nc.dram_tensor("dos", [128, NCORES * TFRAG], BF16,
                                  kind="ExternalOutput"),
        }

    with tile.TileContext(nc) as tc, ExitStack() as ctx:
        ep = ctx.enter_context          # shorthand

        consts = ep(tc.tile_pool(name="consts", bufs=1))
        p_xt = ep(tc.tile_pool(name="xt", bufs=3))
        p_qk = ep(tc.tile_pool(name="qkt", bufs=2))
        p_vt = ep(tc.tile_pool(name="vt", bufs=2))
        p_v = ep(tc.tile_pool(name="vnat", bufs=2 * NJ))
        p_exp = ep(tc.tile_pool(name="exps", bufs=3))
        p_outt = ep(tc.tile_pool(name="outt", bufs=1))
        p_ouc = ep(tc.tile_pool(name="ouc", bufs=6))
        p_small = ep(tc.tile_pool(name="small", bufs=2))
        p_ots = ep(tc.tile_pool(name="ots", bufs=2))
        p_y = ep(tc.tile_pool(name="ysb", bufs=2))
        ps_mm = ep(tc.tile_pool(name="psmm", bufs=2, space="PSUM"))
        ps_sa = ep(tc.tile_pool(name="pssa", bufs=1, space="PSUM"))
        ps_sb = ep(tc.tile_pool(name="pssb", bufs=1, space="PSUM"))
        ps_ou = ep(tc.tile_pool(name="psou", bufs=2, space="PSUM"))

        # ---- constants / weights to SBUF ----
        ident = consts.tile([128, 128], BF16)
        make_identity(nc, ident[:])

        # DMA priority order: qkv weights + first x tile feed the first
        # compute; wp / later x tiles stream in behind them.
        wqk_sb = consts.tile([128, NKC * 256], BF16)
        wv_sb = consts.tile([128, NKC * 128], BF16)
        wp_sb = consts.tile([128, NKC * C], BF16)
        bp_sb = consts.tile([1, C], BF16)
        for c in range(NKC):
            r = slice(c * 128, (c + 1) * 128)
            nc.sync.dma_start(out=wqk_sb[:, c * 256:(c + 1) * 256], in_=wqk_d[r, :])
            nc.scalar.dma_start(out=wv_sb[:, c * 128:(c + 1) * 128], in_=wv_d[r, :])
        nc.sync.dma_start(out=bp_sb[:], in_=bp_d[0:1, :])

        ones_row = consts.tile([1, 128], BF16)
        nc.vector.memset(ones_row[:], 1.0)
        ones_f32 = consts.tile([65, 64], F32)
        nc.vector.memset(ones_f32[:], 1.0)

        # ---- PE warmup: keep HAM at K=8/8 while the x^T DMAs land ----
        for w in range(48):
            wps = ps_mm.tile([128, 128], F32, tag="mm", name="warm")
            nc.tensor.matmul(wps[:], ident[:], ident[:], start=True, stop=True)

        # ---- x^T loads (plain DMA, host pre-transposed), 2 HWDGE queues ----
        xtp = [p_xt.tile([128, NKC * 1024], BF16, tag="xt", name="xti")
               for _ in range(4)]

        def load_x(idx):
            b, tp = idx // 2, idx % 2
            t0 = b * NSEQ + tp * 1024
            xti = xtp[idx]
            for c in range(NKC):
                eng = nc.sync if c % 2 == 0 else nc.scalar
                eng.dma_start(
                    out=xti[:, c * 1024:(c + 1) * 1024],
                    in_=xt_d[c * 128:(c + 1) * 128, t0:t0 + 1024])

        load_x(0)
        load_x(1)
        for c in range(NKC):
            nc.scalar.dma_start(out=wp_sb[:, c * C:(c + 1) * C],
                                in_=wp_d[c * 128:(c + 1) * 128, :])
        load_x(2)
        load_x(3)

        # persistent / cross-stage state
        outT = [p_outt.tile([64, T], BF16, tag=f"outT{h}", name=f"outT{h}")
                for h in range(2)]
        qkT = {}      # b -> (qT, kT)  single [128, NSEQ] tiles
        vTs = {}      # b -> vT
        vns = {}      # b -> [vn tiles]
        outUc = {}    # (b, i, h) -> flushed out_u SBUF tile

        def xts(b, tt, c):
            xti = xtp[b * 2 + tt // 2]
            off = c * 1024 + (tt % 2) * ITILE
            return xti[:, off: off + ITILE]

        def qkv_ensure(b):
            if b not in qkT:
                qz = [p_qk.tile([128, NSEQ], BF16, tag=f"qz{h}", name="qz")
                      for h in range(2)]
                kz = [p_qk.tile([128, NSEQ], BF16, tag=f"kz{h}", name="kz")
                      for h in range(2)]
                # zero the dead half so K=128 contraction ignores the other
                # head (full-array matmuls keep the PE clock un-throttled)
                nc.vector.memset(qz[0][64:128, :], 0.0)
                nc.vector.memset(kz[0][64:128, :], 0.0)
                nc.vector.memset(qz[1][0:64, :], 0.0)
                nc.vector.memset(kz[1][0:64, :], 0.0)
                vT = p_vt.tile([128, NSEQ], BF16, tag="vT", name="vT")
                qkT[b] = (qz, kz)
                vTs[b] = vT

        def qkv_half(b, tt, w):
            # one [128, 512] output tile of q/k/v for token half-tile tt
            qkv_ensure(b)
            qz, kz = qkT[b]
            vT = vTs[b]
            pst = ps_mm.tile([128, ITILE], F32, tag="mm", name="pst")
            for c in range(NKC):
                if w < 2:
                    lhsT = wqk_sb[:, c * 256 + w * 128: c * 256 + (w + 1) * 128]
                else:
                    lhsT = wv_sb[:, c * 128:(c + 1) * 128]
                nc.tensor.matmul(pst[:], lhsT, xts(b, tt, c),
                                 start=(c == 0), stop=(c == NKC - 1))
            tsl2 = slice(tt * ITILE, (tt + 1) * ITILE)
            if w == 2:
                nc.vector.tensor_copy(vT[:, tsl2], pst[:])
            else:
                dst = qz if w == 0 else kz
                nc.vector.tensor_copy(dst[0][0:64, tsl2], pst[0:64, :])
                nc.vector.tensor_copy(dst[1][64:128, tsl2], pst[64:128, :])

        def qkv_tp(b, tp):
            for w in range(3):
                for u in range(2):
                    qkv_half(b, 2 * tp + u, w)

        def vn_block(b, tcns):
            vT = vTs[b]
            vn = vns.setdefault(b, [None] * NJ)
            for tcn in tcns:
                vtr = ps_mm.tile([128, 2 * ITILE], BF16, tag="mm", name="vtr")
                nc.tensor.transpose(vtr[:, 0:128],
                                    vT[:, tcn * 128:(tcn + 1) * 128], ident[:])
                vni = p_v.tile([128, 193], BF16, tag="v", name="vni")
                nc.vector.memset(vni[:, 64:65], 1.0)
                nc.vector.memset(vni[:, 129:130], 1.0)
                nc.vector.memset(vni[:, 130:193], 0.0)
                nc.vector.tensor_copy(vni[:, 0:64], vtr[:, 0:64])
                nc.vector.tensor_copy(vni[:, 65:129], vtr[:, 64:128])
                vn[tcn] = vni

        attn_outu = {}
        filler = []

        def drain_filler(n):
            for _ in range(n):
                if filler:
                    filler.pop(0)()

        def attn_part(b, i, groups):
            qz, kz = qkT[b]
            vn = vns[b]
            isl = slice(i * ITILE, (i + 1) * ITILE)
            if (b, i) not in attn_outu:
                attn_outu[(b, i)] = [ps_ou.tile([128, ITILE], F32, tag="outu",
                                                name="outu")
                                     for _ in range(2)]
            outu = attn_outu[(b, i)]
            for g in groups:
                s_tiles = [ps_sa.tile([128, JG * ITILE], F32, tag="s", name="sa"),
                           ps_sb.tile([128, JG * ITILE], F32, tag="s", name="sb")]
                for k in range(JG):
                    j = g * JG + k
                    jsl = slice(j * 128, (j + 1) * 128)
                    ksl = slice(k * ITILE, (k + 1) * ITILE)
                    for h in range(2):
                        nc.tensor.matmul(
                            s_tiles[h][:, ksl], kz[h][:, jsl], qz[h][:, isl],
                            start=True, stop=True)
                for h in range(2):
                    ex = p_exp.tile([128, JG * ITILE], BF16, tag=f"ex{h}",
                                    name="ex")
                    nc.scalar.activation(ex[:], s_tiles[h][:], EXP, scale=SCALE)
                    if DEBUG and (b, i, g, h) == (0, 0, 0, 0):
                        nc.sync.dma_start(out=dbg["dex"][:, :], in_=ex[:])
                    for k in range(JG):
                        j = g * JG + k
                        nc.tensor.matmul(
                            outu[h][:],
                            vn[j][:, h * 65: h * 65 + 128],
                            ex[:, k * ITILE:(k + 1) * ITILE],
                            start=(j == 0), stop=(j == NJ - 1))
                drain_filler(1)

        def attn_flush(b, i):
            outu = attn_outu.pop((b, i))
            for h in range(2):
                ouc = p_ouc.tile([65, ITILE], F32, tag="ouc", name="ouc")
                nc.vector.tensor_copy(ouc[:], outu[h][0:65, :])
                if DEBUG and (b, i, h) == (0, 0, 0):
                    nc.sync.dma_start(out=dbg["dou"][:, :], in_=ouc[:])
                outUc[(b, i, h)] = ouc

        def attn(b, i):
            attn_part(b, i, range(NJ // JG))
            attn_flush(b, i)

        def norm(b, i):
            t0 = b * NSEQ
            for h in range(2):
                ouc = outUc.pop((b, i, h))
                bc_ps = ps_mm.tile([64, ITILE], F32, tag="mm", name="bcps")
                nc.tensor.matmul(bc_ps[:], ones_f32[64:65, :], ouc[64:65, :],
                                 start=True, stop=True)
                rcp = p_small.tile([64, ITILE], F32, tag=f"rcp{h}", name="rcp")
                nc.vector.reciprocal_approx_fast(rcp[:], bc_ps[:])
                nc.vector.tensor_mul(outT[h][0:64, t0 + i * ITILE:
                                               t0 + (i + 1) * ITILE],
                                     ouc[0:64, :], rcp[:])

        def stage_a2a(q):
            b, t0, nt = STAGES[q]
            hs = b * NSEQ + t0
            for h in range(2):
                dst = a2a_in[q].rearrange("(s hh r) t -> hh r s t",
                                          s=NCORES, hh=2, r=64)[h]
                src = outT[h][0:64, hs:hs + nt].rearrange(
                    "r (s t) -> r s t", s=NCORES)
                nc.sync.dma_start(out=dst, in_=src)
            nc.gpsimd.collective_compute(
                "AllToAll", mybir.AluOpType.bypass,
                replica_groups=[list(range(NCORES))],
                ins=[a2a_in[q][:, :]], outs=[a2a_out[q][:, :]])

        def oproj_quanta(q):
            b, t0, nt = STAGES[q]
            frag = nt // NCORES
            st = {}

            def half_n(n):
                if n == 0:
                    ots = p_ots.tile([128, nt], BF16, tag="ots", name="ots")
                    nc.sync.dma_start(
                        out=ots[:].rearrange("p (s t) -> p s t", s=NCORES),
                        in_=a2a_out[q].rearrange("(s p) t -> p s t", s=NCORES))
                    if DEBUG and q == 0:
                        nc.sync.dma_start(out=dbg["dos"][:, :], in_=ots[:])
                    st["ots"] = ots
                    st["y"] = p_y.tile([128, C], F32, tag="y", name="ysb")
                ots, y_sb = st["ots"], st["y"]
                y_ps = ps_mm.tile([128, ITILE], F32, tag="mm", name="yps")
                for s in range(NKC):
                    nc.tensor.matmul(
                        y_ps[0:frag, :],
                        ots[:, s * frag:(s + 1) * frag],
                        wp_sb[:, s * C + n * ITILE: s * C + (n + 1) * ITILE],
                        start=(s == 0), stop=False)
                nc.tensor.matmul(y_ps[0:frag, :], ones_row[:, 0:frag],
                                 bp_sb[:, n * ITILE:(n + 1) * ITILE],
                                 start=False, stop=True)
                nc.vector.tensor_copy(y_sb[0:frag, n * ITILE:(n + 1) * ITILE],
                                      y_ps[0:frag, :])
                if n == 1:
                    yr0 = b * (TSL // B) + t0 // NCORES
                    nc.sync.dma_start(out=y_d[yr0: yr0 + frag, :],
                                      in_=y_sb[0:frag, :])

            return [lambda: half_n(0), lambda: half_n(1)]

        def warm_fill(n):
            for _ in range(n):
                wps = ps_mm.tile([128, 128], F32, tag="mm", name="warm2")
                nc.tensor.matmul(wps[:], ident[:], ident[:], start=True,
                                 stop=True)

        def qkv_quanta(b):
            qs = []
            for tp in range(2):
                for w in range(3):
                    for u in range(2):
                        qs.append(lambda b=b, tt=2 * tp + u, w=w:
                                  qkv_half(b, tt, w))
            return qs

        def vn_quanta(b):
            return [lambda b=b, t=t: vn_block(b, range(2 * t, 2 * t + 2))
                    for t in range(NJ // 2)]

        # ---- software-pipelined emission schedule ----
        qkv_ensure(0)
        qkv_ensure(1)     # zero-halves memset on DVE during the DMA head
        qkv_tp(0, 0)
        vn_block(0, range(NJ // 2))
        attn_part(0, 0, range(NJ // JG // 2))
        qkv_tp(0, 1)
        vn_block(0, range(NJ // 2, NJ))
        if DEBUG:
            qz0, kz0 = qkT[0]
            nc.sync.dma_start(out=dbg["dqz"][:, :], in_=qz0[0][:])
            nc.sync.dma_start(out=dbg["dkz"][:, :], in_=kz0[0][:])
            nc.sync.dma_start(out=dbg["dvt"][:, :], in_=vTs[0][:])
            nc.sync.dma_start(out=dbg["dvn"][:, :], in_=vns[0][0][:, 0:130])
        filler += qkv_quanta(1) + vn_quanta(1)
        attn_part(0, 0, range(NJ // JG // 2, NJ // JG)); attn_flush(0, 0)
        attn(0, 1); norm(0, 0); norm(0, 1); stage_a2a(0)
        if DEBUG:
            nc.sync.dma_start(out=dbg["dot"][:, :],
                              in_=outT[0][0:64, 0:NSEQ])
        attn(0, 2)
        filler += oproj_quanta(0)
        attn(0, 3); norm(0, 2); norm(0, 3); stage_a2a(1)
        filler += oproj_quanta(1)
        attn(1, 0)
        attn(1, 1); norm(1, 0); norm(1, 1); stage_a2a(2)
        filler += oproj_quanta(2)
        attn(1, 2); norm(1, 2); stage_a2a(3)
        filler += oproj_quanta(3)
        attn(1, 3); norm(1, 3); stage_a2a(4)
        drain_filler(len(filler))
        warm_fill(24)
        for f in oproj_quanta(4):
            f()

    nc.compile()
    return nc


_NC = None


def _get_nc():
    global _NC
    if _NC is None:
        _NC = build_program()
    return _NC


def prep_in_maps(x, w_qkv, w_proj, b_proj):
    x_bf = np.ascontiguousarray(np.asarray(x, dtype=np.float32).reshape(T, C)
                                .T).astype(NPBF16)          # [C, T] host side
    w_qkv = np.asarray(w_qkv, dtype=np.float32)
    w_proj = np.asarray(w_proj, dtype=np.float32)
    b_proj = np.asarray(b_proj, dtype=np.float32)
    wp_bf = np.ascontiguousarray(w_proj).astype(NPBF16)
    bp_bf = b_proj.reshape(1, C).astype(NPBF16)

    q_w, k_w, v_w = w_qkv[:, 0:C], w_qkv[:, C:2 * C], w_qkv[:, 2 * C:3 * C]
    in_maps = []
    for c in range(NCORES):
        hA, hB = 2 * c, 2 * c + 1
        sA, sB = slice(hA * D, (hA + 1) * D), slice(hB * D, (hB + 1) * D)
        wqk_c = np.concatenate([q_w[:, sA], q_w[:, sB], k_w[:, sA], k_w[:, sB]],
                               axis=1).astype(NPBF16)
        wv_c = np.concatenate([v_w[:, sA], v_w[:, sB]], axis=1).astype(NPBF16)
        in_maps.append({"xt": x_bf, "wqk": np.ascontiguousarray(wqk_c),
                        "wv": np.ascontiguousarray(wv_c), "wproj": wp_bf,
                        "bproj": bp_bf})
    return in_maps


def assemble(results):
    y = np.empty((T, C), dtype=np.float32)
    for c in range(NCORES):
        yc = results[c]["y"]
        for b, t0, nt in STAGES:
            frag = nt // NCORES
            g0 = b * NSEQ + t0 + c * frag
            r0 = b * (TSL // B) + t0 // NCORES
            y[g0: g0 + frag, :] = yc[r0: r0 + frag, :]
    return y.reshape(B, NSEQ, C)


def run(in_maps, trace=False):
    nc = _get_nc()
    return run_bass_kernel_spmd(nc, in_maps, core_ids=list(range(NCORES)),
                                trace=trace)


def kernel(x, w_qkv, w_proj, b_proj):
    res = run(prep_in_maps(x, w_qkv, w_proj, b_proj))
    return assemble(res.results)


# revision 36
# speedup vs baseline: 1.0576x; 1.0576x over previous
"""Multi-head attention (B=2, N=2048, C=1024, H=16) on 8 Trainium2 NeuronCores.

Sharding: tensor-parallel over heads (2 heads/core) for qkv-proj + attention;
all-to-all of the attention output (split into 4 half-batch collectives,
pipelined under attention), then each core runs the output projection over
the full channel dim for its token slices.  Host concatenates slices.

v2 changes vs baseline:
  - x is transposed on the HOST: xt input is [C, T] bf16, loaded with plain
    DMAs (split across the two HWDGE queues) -- no on-device DMA-transposes.
  - PE warmup stream at t=0 so HAM is at K=8/8 when real matmuls start.
  - q^T/k^T stored as single [128, tok] tiles (head A rows 0:64, head B rows
    64:128, the natural qkv-matmul output); the S^T matmuls are issued as
    K=64 pairs on disjoint PE row-groups (tile_position auto-derived), so
    both heads' S run CONCURRENTLY on the array halves -> S cost halves.
  - normalization uses reciprocal_approx_fast (~5x faster than reciprocal)
    and multiplies against the PSUM broadcast directly.
  - a2a staging/gather DMAs consolidated via rearrange APs (2+1 per stage
    instead of 16+8).

Device layouts (per core, heads A=2c, B=2c+1):
  x^T  [c_in, tok]          8 chunk tiles, plain DMA from host-transposed xt
  q^T/k^T [128, tok]        rows 0-63 head A, 64-127 head B
  v^T  [128, tok]           -> PE-transposed per 128-chunk into vn [128,130]
                            cols: [v_A|1|v_B|1]  (ones col -> softmax denom)
  S^T  [j, i] = k^T.T @ q^T per head (K=64 concurrent row-group pair)
  expS = exp(0.125 * S^T)   (ScalarE, bf16)
  out_u^T [65, i] = [v|1].T @ expS  (rows 0-63 out, row 64 = denominator)
  normalize via DVE reciprocal_approx_fast + fp32 K=1 broadcast matmul
"""

import numpy as np
import ml_dtypes
from contextlib import ExitStack

import concourse.bass as bass
import concourse.tile as tile
from concourse import bacc, mybir
from concourse.bass_utils import run_bass_kernel_spmd
from concourse.masks import make_identity

BF16 = mybir.dt.bfloat16
F32 = mybir.dt.float32
EXP = mybir.ActivationFunctionType.Exp
NPBF16 = ml_dtypes.bfloat16

NCORES = 8
B, NSEQ, C, H, D = 2, 2048, 1024, 16, 64
T = B * NSEQ                 # 4096 flattened tokens
SCALE = D ** -0.5            # folded into the exp activation
NKC = C // 128               # 8 contraction chunks
ITILE = 512                  # query tile (free dim of S^T)
NI = NSEQ // ITILE           # 4 i-tiles per batch
NJ = NSEQ // 128             # 16 key chunks per batch
JG = 2                       # j-chunks per exp group ([128,1024] psum tiles)
HALF = 1024                  # tokens per all-to-all (half batch)
TFRAG = HALF // NCORES       # 128 tokens per core per all-to-all
TSL = B * NSEQ // NCORES     # 512 output tokens per core

# a2a stages: (batch, start token within batch, n tokens); the final stages
# are small so the last collective's latency tail is short.
STAGES = [(0, 0, 1024), (0, 1024, 1024), (1, 0, 1024),
          (1, 1024, 512), (1, 1536, 512)]

DEBUG = False


def build_program():
    nc = bacc.Bacc("TRN2", target_bir_lowering=False, debug=False,
                   num_devices=NCORES)

    xt_d = nc.dram_tensor("xt", [C, T], BF16, kind="ExternalInput")
    wqk_d = nc.dram_tensor("wqk", [C, 256], BF16, kind="ExternalInput")
    wv_d = nc.dram_tensor("wv", [C, 128], BF16, kind="ExternalInput")
    wp_d = nc.dram_tensor("wproj", [C, C], BF16, kind="ExternalInput")
    bp_d = nc.dram_tensor("bproj", [1, C], BF16, kind="ExternalInput")
    y_d = nc.dram_tensor("y", [TSL, C], F32, kind="ExternalOutput")

    ccw_in = nc.dram_tensor("ccw_in", [NCORES, 8], BF16)
    ccw_out = nc.dram_tensor("ccw_out", [NCORES, 8], BF16)
    a2a_in = [nc.dram_tensor(f"a2a_in{q}", [NCORES * 128, nt // NCORES], BF16)
              for q, (_, _, nt) in enumerate(STAGES)]
    a2a_out = [nc.dram_tensor(f"a2a_out{q}", [NCORES * 128, nt // NCORES],
                              BF16)
               for q, (_, _, nt) in enumerate(STAGES)]
    if DEBUG:
        dbg = {
            "dqz": nc.dram_tensor("dqz", [128, NSEQ], BF16,
                                  kind="ExternalOutput"),
            "dkz": nc.dram_tensor("dkz", [128, NSEQ], BF16,
                                  kind="ExternalOutput"),
            "dvt": nc.dram_tensor("dvt", [128, NSEQ], BF16,
                                  kind="ExternalOutput"),
            "dvn": nc.dram_tensor("dvn", [128, 130], BF16,
                                  kind="ExternalOutput"),
            "dex": nc.dram_tensor("dex", [128, 2 * ITILE], BF16,
                                  kind="ExternalOutput"),
            "dou": nc.dram_tensor("dou", [65, ITILE], F32,
                                  kind="ExternalOutput"),
            "dot": nc.dram_tensor("dot", [64, NSEQ], BF16,
                                  kind="ExternalOutput"),
            "dos": nc.dram_tensor("dos", [128, NCORES * TFRAG], BF16,
                                  kind="ExternalOutput"),
        }

    with tile.TileContext(nc) as tc, ExitStack() as ctx:
        ep = ctx.enter_context          # shorthand

        consts = ep(tc.tile_pool(name="consts", bufs=1))
        p_xt = ep(tc.tile_pool(name="xt", bufs=3))
        p_qk = ep(tc.tile_pool(name="qkt", bufs=2))
        p_vt = ep(tc.tile_pool(name="vt", bufs=2))
        p_v = ep(tc.tile_pool(name="vnat", bufs=2 * NJ))
        p_exp = ep(tc.tile_pool(name="exps", bufs=3))
        p_outt = ep(tc.tile_pool(name="outt", bufs=1))
        p_ouc = ep(tc.tile_pool(name="ouc", bufs=6))
        p_small = ep(tc.tile_pool(name="small", bufs=2))
        p_ots = ep(tc.tile_pool(name="ots", bufs=2))
        p_y = ep(tc.tile_pool(name="ysb", bufs=2))
        ps_mm = ep(tc.tile_pool(name="psmm", bufs=2, space="PSUM"))
        ps_sa = ep(tc.tile_pool(name="pssa", bufs=1, space="PSUM"))
        ps_sb = ep(tc.tile_pool(name="pssb", bufs=1, space="PSUM"))
        ps_ou = ep(tc.tile_pool(name="psou", bufs=2, space="PSUM"))

        # ---- constants / weights to SBUF ----
        ident = consts.tile([128, 128], BF16)
        make_identity(nc, ident[:])

        # DMA priority order: qkv weights + first x tile feed the first
        # compute; wp / later x tiles stream in behind them.
        wqk_sb = consts.tile([128, NKC * 256], BF16)
        wv_sb = consts.tile([128, NKC * 128], BF16)
        wp_sb = consts.tile([128, NKC * C], BF16)
        bp_sb = consts.tile([1, C], BF16)
        for c in range(NKC):
            r = slice(c * 128, (c + 1) * 128)
            nc.sync.dma_start(out=wqk_sb[:, c * 256:(c + 1) * 256], in_=wqk_d[r, :])
            nc.scalar.dma_start(out=wv_sb[:, c * 128:(c + 1) * 128], in_=wv_d[r, :])
        nc.sync.dma_start(out=bp_sb[:], in_=bp_d[0:1, :])

        ones_row = consts.tile([1, 128], BF16)
        nc.vector.memset(ones_row[:], 1.0)
        ones_f32 = consts.tile([65, 64], F32)
        nc.vector.memset(ones_f32[:], 1.0)

        # warm-up collective: absorbs ncfw cold-start + inter-core skew
        # early, while the PE is busy with qkv, so the first real a2a
        # completes at data latency instead of ~35us.
        ccw_sb = consts.tile([NCORES, 8], BF16)
        nc.vector.memset(ccw_sb[:], 0.0)
        nc.sync.dma_start(out=ccw_in[:, :], in_=ccw_sb[:])
        nc.gpsimd.collective_compute(
            "AllToAll", mybir.AluOpType.bypass,
            replica_groups=[list(range(NCORES))],
            ins=[ccw_in[:, :]], outs=[ccw_out[:, :]])

        # ---- PE warmup: keep HAM at K=8/8 while the x^T DMAs land ----
        for w in range(48):
            wps = ps_mm.tile([128, 128], F32, tag="mm", name="warm")
            nc.tensor.matmul(wps[:], ident[:], ident[:], start=True, stop=True)

        # ---- x^T loads (plain DMA, host pre-transposed), 2 HWDGE queues ----
        xtp = [p_xt.tile([128, NKC * 1024], BF16, tag="xt", name="xti")
               for _ in range(4)]

        def load_x(idx):
            b, tp = idx // 2, idx % 2
            t0 = b * NSEQ + tp * 1024
            xti = xtp[idx]
            for c in range(NKC):
                eng = nc.sync if c % 2 == 0 else nc.scalar
                eng.dma_start(
                    out=xti[:, c * 1024:(c + 1) * 1024],
                    in_=xt_d[c * 128:(c + 1) * 128, t0:t0 + 1024])

        load_x(0)
        load_x(1)
        for c in range(NKC):
            nc.scalar.dma_start(out=wp_sb[:, c * C:(c + 1) * C],
                                in_=wp_d[c * 128:(c + 1) * 128, :])
        load_x(2)
        load_x(3)

        # persistent / cross-stage state
        outT = [p_outt.tile([64, T], BF16, tag=f"outT{h}", name=f"outT{h}")
                for h in range(2)]
        qkT = {}      # b -> (qT, kT)  single [128, NSEQ] tiles
        vTs = {}      # b -> vT
        vns = {}      # b -> [vn tiles]
        outUc = {}    # (b, i, h) -> flushed out_u SBUF tile

        def xts(b, tt, c):
            xti = xtp[b * 2 + tt // 2]
            off = c * 1024 + (tt % 2) * ITILE
            return xti[:, off: off + ITILE]

        def qkv_ensure(b):
            if b not in qkT:
                qz = [p_qk.tile([128, NSEQ], BF16, tag=f"qz{h}", name="qz")
                      for h in range(2)]
                kz = [p_qk.tile([128, NSEQ], BF16, tag=f"kz{h}", name="kz")
                      for h in range(2)]
                # zero the dead half so K=128 contraction ignores the other
                # head (full-array matmuls keep the PE clock un-throttled)
                nc.vector.memset(qz[0][64:128, :], 0.0)
                nc.vector.memset(kz[0][64:128, :], 0.0)
                nc.vector.memset(qz[1][0:64, :], 0.0)
                nc.vector.memset(kz[1][0:64, :], 0.0)
                vT = p_vt.tile([128, NSEQ], BF16, tag="vT", name="vT")
                qkT[b] = (qz, kz)
                vTs[b] = vT

        def qkv_half(b, tt, w):
            # one [128, 512] output tile of q/k/v for token half-tile tt
            qkv_ensure(b)
            qz, kz = qkT[b]
            vT = vTs[b]
            pst = ps_mm.tile([128, ITILE], F32, tag="mm", name="pst")
            for c in range(NKC):
                if w < 2:
                    lhsT = wqk_sb[:, c * 256 + w * 128: c * 256 + (w + 1) * 128]
                else:
                    lhsT = wv_sb[:, c * 128:(c + 1) * 128]
                nc.tensor.matmul(pst[:], lhsT, xts(b, tt, c),
                                 start=(c == 0), stop=(c == NKC - 1))
            tsl2 = slice(tt * ITILE, (tt + 1) * ITILE)
            if w == 2:
                nc.vector.tensor_copy(vT[:, tsl2], pst[:])
            else:
                dst = qz if w == 0 else kz
                nc.vector.tensor_copy(dst[0][0:64, tsl2], pst[0:64, :])
                nc.vector.tensor_copy(dst[1][64:128, tsl2], pst[64:128, :])

        def qkv_tp(b, tp):
            for w in range(3):
                for u in range(2):
                    qkv_half(b, 2 * tp + u, w)

        def vn_block(b, tcns):
            vT = vTs[b]
            vn = vns.setdefault(b, [None] * NJ)
            for tcn in tcns:
                vtr = ps_mm.tile([128, 2 * ITILE], BF16, tag="mm", name="vtr")
                nc.tensor.transpose(vtr[:, 0:128],
                                    vT[:, tcn * 128:(tcn + 1) * 128], ident[:])
                vni = p_v.tile([128, 193], BF16, tag="v", name="vni")
                nc.vector.memset(vni[:, 64:65], 1.0)
                nc.vector.memset(vni[:, 129:130], 1.0)
                nc.vector.memset(vni[:, 130:193], 0.0)
                nc.vector.tensor_copy(vni[:, 0:64], vtr[:, 0:64])
                nc.vector.tensor_copy(vni[:, 65:129], vtr[:, 64:128])
                vn[tcn] = vni

        attn_outu = {}
        filler = []

        def drain_filler(n):
            for _ in range(n):
                if filler:
                    filler.pop(0)()

        def attn_part(b, i, groups):
            qz, kz = qkT[b]
            vn = vns[b]
            isl = slice(i * ITILE, (i + 1) * ITILE)
            if (b, i) not in attn_outu:
                attn_outu[(b, i)] = [ps_ou.tile([128, ITILE], F32, tag="outu",
                                                name="outu")
                                     for _ in range(2)]
            outu = attn_outu[(b, i)]
            for g in groups:
                s_tiles = [ps_sa.tile([128, JG * ITILE], F32, tag="s", name="sa"),
                           ps_sb.tile([128, JG * ITILE], F32, tag="s", name="sb")]
                for k in range(JG):
                    j = g * JG + k
                    jsl = slice(j * 128, (j + 1) * 128)
                    ksl = slice(k * ITILE, (k + 1) * ITILE)
                    for h in range(2):
                        nc.tensor.matmul(
                            s_tiles[h][:, ksl], kz[h][:, jsl], qz[h][:, isl],
                            start=True, stop=True)
                for h in range(2):
                    ex = p_exp.tile([128, JG * ITILE], BF16, tag=f"ex{h}",
                                    name="ex")
                    nc.scalar.activation(ex[:], s_tiles[h][:], EXP, scale=SCALE)
                    if DEBUG and (b, i, g, h) == (0, 0, 0, 0):
                        nc.sync.dma_start(out=dbg["dex"][:, :], in_=ex[:])
                    for k in range(JG):
                        j = g * JG + k
                        nc.tensor.matmul(
                            outu[h][:],
                            vn[j][:, h * 65: h * 65 + 128],
                            ex[:, k * ITILE:(k + 1) * ITILE],
                            start=(j == 0), stop=(j == NJ - 1))
                drain_filler(1)

        def attn_flush(b, i):
            outu = attn_outu.pop((b, i))
            for h in range(2):
                ouc = p_ouc.tile([65, ITILE], F32, tag="ouc", name="ouc")
                nc.vector.tensor_copy(ouc[:], outu[h][0:65, :])
                if DEBUG and (b, i, h) == (0, 0, 0):
                    nc.sync.dma_start(out=dbg["dou"][:, :], in_=ouc[:])
                outUc[(b, i, h)] = ouc

        def attn(b, i):
            attn_part(b, i, range(NJ // JG))
            attn_flush(b, i)

        def norm(b, i):
            t0 = b * NSEQ
            for h in range(2):
                ouc = outUc.pop((b, i, h))
                bc_ps = ps_mm.tile([64, ITILE], F32, tag="mm", name="bcps")
                nc.tensor.matmul(bc_ps[:], ones_f32[64:65, :], ouc[64:65, :],
                                 start=True, stop=True)
                rcp = p_small.tile([64, ITILE], F32, tag=f"rcp{h}", name="rcp")
                nc.vector.reciprocal_approx_fast(rcp[:], bc_ps[:])
                nc.vector.tensor_mul(outT[h][0:64, t0 + i * ITILE:
                                               t0 + (i + 1) * ITILE],
                                     ouc[0:64, :], rcp[:])

        def stage_a2a(q):
            b, t0, nt = STAGES[q]
            hs = b * NSEQ + t0
            for h in range(2):
                dst = a2a_in[q].rearrange("(s hh r) t -> hh r s t",
                                          s=NCORES, hh=2, r=64)[h]
                src = outT[h][0:64, hs:hs + nt].rearrange(
                    "r (s t) -> r s t", s=NCORES)
                nc.sync.dma_start(out=dst, in_=src)
            nc.gpsimd.collective_compute(
                "AllToAll", mybir.AluOpType.bypass,
                replica_groups=[list(range(NCORES))],
                ins=[a2a_in[q][:, :]], outs=[a2a_out[q][:, :]])

        def oproj_quanta(q):
            b, t0, nt = STAGES[q]
            frag = nt // NCORES
            st = {}

            def half_n(n):
                if n == 0:
                    ots = p_ots.tile([128, nt], BF16, tag="ots", name="ots")
                    nc.sync.dma_start(
                        out=ots[:].rearrange("p (s t) -> p s t", s=NCORES),
                        in_=a2a_out[q].rearrange("(s p) t -> p s t", s=NCORES))
                    if DEBUG and q == 0:
                        nc.sync.dma_start(out=dbg["dos"][:, :], in_=ots[:])
                    st["ots"] = ots
                    st["y"] = p_y.tile([128, C], F32, tag="y", name="ysb")
                ots, y_sb = st["ots"], st["y"]
                y_ps = ps_mm.tile([128, ITILE], F32, tag="mm", name="yps")
                for s in range(NKC):
                    nc.tensor.matmul(
                        y_ps[0:frag, :],
                        ots[:, s * frag:(s + 1) * frag],
                        wp_sb[:, s * C + n * ITILE: s * C + (n + 1) * ITILE],
                        start=(s == 0), stop=False)
                nc.tensor.matmul(y_ps[0:frag, :], ones_row[:, 0:frag],
                                 bp_sb[:, n * ITILE:(n + 1) * ITILE],
                                 start=False, stop=True)
                nc.vector.tensor_copy(y_sb[0:frag, n * ITILE:(n + 1) * ITILE],
                                      y_ps[0:frag, :])
                if n == 1:
                    yr0 = b * (TSL // B) + t0 // NCORES
                    nc.sync.dma_start(out=y_d[yr0: yr0 + frag, :],
                                      in_=y_sb[0:frag, :])

            return [lambda: half_n(0), lambda: half_n(1)]

        def warm_fill(n):
            for _ in range(n):
                wps = ps_mm.tile([128, 128], F32, tag="mm", name="warm2")
                nc.tensor.matmul(wps[:], ident[:], ident[:], start=True,
                                 stop=True)

        def qkv_quanta(b, tps=(0, 1)):
            qs = []
            for tp in tps:
                for w in range(3):
                    for u in range(2):
                        qs.append(lambda b=b, tt=2 * tp + u, w=w:
                                  qkv_half(b, tt, w))
            return qs

        def vn_quanta(b):
            return [lambda b=b, t=t: vn_block(b, range(2 * t, 2 * t + 2))
                    for t in range(NJ // 2)]

        # ---- software-pipelined emission schedule ----
        qkv_ensure(0)
        qkv_ensure(1)     # zero-halves memset on DVE during the DMA head
        qkv_tp(0, 0)
        vn_block(0, range(NJ // 2))
        filler += qkv_quanta(0, tps=(1,))
        attn_part(0, 0, range(NJ // JG // 2))
        drain_filler(len(filler))
        vn_block(0, range(NJ // 2, NJ))
        if DEBUG:
            qz0, kz0 = qkT[0]
            nc.sync.dma_start(out=dbg["dqz"][:, :], in_=qz0[0][:])
            nc.sync.dma_start(out=dbg["dkz"][:, :], in_=kz0[0][:])
            nc.sync.dma_start(out=dbg["dvt"][:, :], in_=vTs[0][:])
            nc.sync.dma_start(out=dbg["dvn"][:, :], in_=vns[0][0][:, 0:130])
        filler += qkv_quanta(1) + vn_quanta(1)
        attn_part(0, 0, range(NJ // JG // 2, NJ // JG)); attn_flush(0, 0)
        attn(0, 1); norm(0, 0); norm(0, 1); stage_a2a(0)
        if DEBUG:
            nc.sync.dma_start(out=dbg["dot"][:, :],
                              in_=outT[0][0:64, 0:NSEQ])
        attn(0, 2)
        attn(0, 3); norm(0, 2); norm(0, 3); stage_a2a(1)
        filler += oproj_quanta(0)
        attn(1, 0)
        filler += oproj_quanta(1)
        attn(1, 1); norm(1, 0); norm(1, 1); stage_a2a(2)
        attn(1, 2); norm(1, 2); stage_a2a(3)
        filler += oproj_quanta(2)
        attn(1, 3); norm(1, 3); stage_a2a(4)
        drain_filler(len(filler))
        for f in oproj_quanta(3):
            f()
        warm_fill(60)
        for f in oproj_quanta(4):
            f()

    nc.compile()
    return nc


_NC = None


def _get_nc():
    global _NC
    if _NC is None:
        _NC = build_program()
    return _NC


def prep_in_maps(x, w_qkv, w_proj, b_proj):
    x_bf = np.ascontiguousarray(np.asarray(x, dtype=np.float32).reshape(T, C)
                                .T).astype(NPBF16)          # [C, T] host side
    w_qkv = np.asarray(w_qkv, dtype=np.float32)
    w_proj = np.asarray(w_proj, dtype=np.float32)
    b_proj = np.asarray(b_proj, dtype=np.float32)
    wp_bf = np.ascontiguousarray(w_proj).astype(NPBF16)
    bp_bf = b_proj.reshape(1, C).astype(NPBF16)

    q_w, k_w, v_w = w_qkv[:, 0:C], w_qkv[:, C:2 * C], w_qkv[:, 2 * C:3 * C]
    in_maps = []
    for c in range(NCORES):
        hA, hB = 2 * c, 2 * c + 1
        sA, sB = slice(hA * D, (hA + 1) * D), slice(hB * D, (hB + 1) * D)
        wqk_c = np.concatenate([q_w[:, sA], q_w[:, sB], k_w[:, sA], k_w[:, sB]],
                               axis=1).astype(NPBF16)
        wv_c = np.concatenate([v_w[:, sA], v_w[:, sB]], axis=1).astype(NPBF16)
        in_maps.append({"xt": x_bf, "wqk": np.ascontiguousarray(wqk_c),
                        "wv": np.ascontiguousarray(wv_c), "wproj": wp_bf,
                        "bproj": bp_bf})
    return in_maps


def assemble(results):
    y = np.empty((T, C), dtype=np.float32)
    for c in range(NCORES):
        yc = results[c]["y"]
        for b, t0, nt in STAGES:
            frag = nt // NCORES
            g0 = b * NSEQ + t0 + c * frag
            r0 = b * (TSL // B) + t0 // NCORES
            y[g0: g0 + frag, :] = yc[r0: r0 + frag, :]
    return y.reshape(B, NSEQ, C)


def run(in_maps, trace=False):
    nc = _get_nc()
    return run_bass_kernel_spmd(nc, in_maps, core_ids=list(range(NCORES)),
                                trace=trace)


def kernel(x, w_qkv, w_proj, b_proj):
    res = run(prep_in_maps(x, w_qkv, w_proj, b_proj))
    return assemble(res.results)
